# revision 11
# baseline (speedup 1.0000x reference)
"""Trainium2 Bass kernel for nn_KANLayer (B=16384, D=1024, K=8).

Math: the per-feature basis chain collapses algebraically:
    nl[b,i] = sum_k (x[b,i]*W1[i,k] + b1[i,k]) * W2[i,k]
            = x[b,i] * a[i] + c[i],   a = sum_k W1*W2, c = sum_k b1*W2
so the whole layer is ONE dense matmul with a fused diagonal + bias:
    out = x @ (lin_W.T + diag(a)) + (lin_b + c)

Precision strategy (validated numerically on the fixed seed-0 inputs,
rel err ~1e-2 vs the 2e-2 gate): split W_eff = W_off + diag(d).
  - x @ W_off runs on the PE in fp8 e4m3 with perf_mode=DoubleRow
    (2 fp8/cell, K=256 per matmul) — ~2x bf16 throughput.
  - the diagonal term d[i]*x[b,i] is large (|d|~1 vs |W_off|~1/32), so
    it is applied at eviction from an fp16 copy of x with one DVE
    scalar_tensor_tensor op per tile (16-bit operands -> 2x DVE rate).
  - output stored bf16 (rounding err ~0.4%, well inside the gate).

Layout: everything transposed — W is the PE-stationary operand, x^T the
moving one, so psum holds out^T[i, b] and the diagonal/bias become
per-partition scalars. Host transposes the output back.

DMA scheduling: per-dma_start transfers land on a single HW queue and
all enqueued transfers fair-share HBM bandwidth, so loads are split
into ~128-256KB pieces issued in priority order (x8 k-pair 0 + all of
w8 first), and the late-needed fp16 x pieces for k-pairs 1-3 are
issued from the ACT instruction stream after earlier evictions so they
cannot starve the critical path. Stores ride the gpsimd ring.

Sharding: data-parallel over batch across 8 NeuronCores (2048 rows
each); weights replicated. No collectives.
"""

from contextlib import ExitStack

import numpy as np
import ml_dtypes

import concourse.bass as bass
import concourse.tile as tile
from concourse import bacc, mybir
from concourse.bass_utils import run_bass_kernel_spmd

B, D = 16384, 1024
NCORES = 8
BS = B // NCORES   # 2048 batch rows per core
P = 128
TP = 4             # contraction k-pairs (each pair = 256 rows via DoubleRow)
IB = D // P        # 8 output-feature blocks of 128
NBC = BS // 512    # 4 batch chunks of 512

FP8 = mybir.dt.float8e4
FP8_NP = ml_dtypes.float8_e4m3fn  # matches TRN fp8e4 within +-240
F16 = mybir.dt.float16

_CACHE = {}


def _build_nc():
    nc = bacc.Bacc("TRN2", target_bir_lowering=False, debug=False,
                   num_devices=NCORES)
    # x8: x^T per core in fp8 (PE moving operand), layout [p, t, bc, 2, b512]
    # with contraction index j = (2*t + sub)*128 + p; each (t, bc) piece is
    # contiguous per partition.
    x8 = nc.dram_tensor("x8", [P, TP, NBC, 2, 512], FP8,
                        kind="ExternalInput").ap()
    # xf: x^T in fp16 for the diagonal correction, same layout.
    xf = nc.dram_tensor("xf", [P, TP, NBC, 2, 512], F16,
                        kind="ExternalInput").ap()
    # W_off (diag zeroed), layout [p, ib, t, 2, i128]: one piece per ib
    # column block holding all k-pairs.
    w8 = nc.dram_tensor("w8", [P, IB, TP, 2, P], FP8,
                        kind="ExternalInput").ap()
    # diag + bias as per-partition columns: dv[p, ib] = d[ib*128+p]
    dv = nc.dram_tensor("dv", [P, IB], mybir.dt.float32,
                        kind="ExternalInput").ap()
    bv = nc.dram_tensor("bv", [P, IB], mybir.dt.float32,
                        kind="ExternalInput").ap()
    # out^T bf16: out[p, ib, b] = result[b, ib*128+p]
    out = nc.dram_tensor("out", [P, IB, BS], mybir.dt.bfloat16,
                         kind="ExternalOutput").ap()

    Act = mybir.ActivationFunctionType
    Alu = mybir.AluOpType
    DR = mybir.MatmulPerfMode.DoubleRow

    with tile.TileContext(nc) as tc, ExitStack() as ctx:
        cpool = ctx.enter_context(tc.tile_pool(name="cpool", bufs=1))
        opool = ctx.enter_context(tc.tile_pool(name="opool", bufs=3))
        tpool = ctx.enter_context(tc.tile_pool(name="tpool", bufs=8))
        ppool = ctx.enter_context(tc.tile_pool(name="ppool", bufs=8,
                                               space="PSUM"))

        # --- priority wave: x8 k-pair 0 (by b-chunk) + all of w8 ---
        x_t = [cpool.tile([P, NBC, 2, 512], FP8, tag=f"x{t}", name=f"x_t{t}")
               for t in range(TP)]
        for bc in range(NBC):
            nc.sync.dma_start(out=x_t[0][:, bc], in_=x8[:, 0, bc])
        w_t = cpool.tile([P, IB, TP, 2, P], FP8, tag="w", name="w_t")
        for ib in range(IB):
            nc.scalar.dma_start(out=w_t[:, ib], in_=w8[:, ib])

        dv_t = cpool.tile([P, IB], mybir.dt.float32, tag="dv", name="dv_t")
        bv_t = cpool.tile([P, IB], mybir.dt.float32, tag="bv", name="bv_t")
        nc.sync.dma_start(out=dv_t, in_=dv)
        nc.sync.dma_start(out=bv_t, in_=bv)

        # --- second wave: rest of x8, then fp16 x for k-pair 0 ---
        xf_t = [cpool.tile([P, NBC, 2, 512], F16, tag=f"xf{t}",
                           name=f"xf_t{t}") for t in range(TP)]
        for t in range(1, TP):
            for bc in range(NBC):
                nc.sync.dma_start(out=x_t[t][:, bc], in_=x8[:, t, bc])
        for bc in range(NBC):
            nc.gpsimd.dma_start(out=xf_t[0][:, bc], in_=xf[:, 0, bc])

        # PE pre-warm with fp8 DoubleRow matmuls on a zero tile so the
        # HAM clock-gate ramps while the input DMAs run.
        warm = cpool.tile([P, 2, 512], FP8, tag="warm", name="warm")
        nc.vector.memset(warm, 0.0)
        warm_ps = ppool.tile([P, 512], mybir.dt.float32, tag="ps",
                             name="warm_ps")
        NWARM = 20
        for i in range(NWARM):
            nc.tensor.matmul(warm_ps, lhsT=warm[:, :, :P], rhs=warm,
                             start=(i == 0), stop=(i == NWARM - 1),
                             perf_mode=DR)

        for ib in range(IB):
            psums = [ppool.tile([P, 512], mybir.dt.float32, tag="ps",
                                name=f"ps{ib}_{bc}") for bc in range(NBC)]
            for t in range(TP):
                for bc in range(NBC):
                    nc.tensor.matmul(
                        psums[bc],
                        lhsT=w_t[:, ib, t],
                        rhs=x_t[t][:, bc],
                        start=(t == 0),
                        stop=(t == TP - 1),
                        perf_mode=DR,
                    )
            # eviction: out^T[i,b] = psum + d[i]*x_f16[i,b] + bias[i]
            t8, sub = divmod(ib, 2)
            o_t = opool.tile([P, BS], mybir.dt.bfloat16, tag="o",
                             name=f"o_t{ib}")
            for bc in range(NBC):
                tb = tpool.tile([P, 512], F16, tag="tb",
                                name=f"tb{ib}_{bc}")
                nc.scalar.activation(tb, psums[bc], Act.Identity,
                                     bias=bv_t[:, ib:ib + 1], scale=1.0)
                nc.vector.scalar_tensor_tensor(
                    o_t[:, bass.ts(bc, 512)], in0=xf_t[t8][:, bc, sub],
                    scalar=dv_t[:, ib:ib + 1],
                    in1=tb, op0=Alu.mult, op1=Alu.add)
                if ib == IB - 1:
                    # pipeline the kernel tail: store per 512-chunk
                    nc.gpsimd.dma_start(out=out[:, ib, bass.ts(bc, 512)],
                                        in_=o_t[:, bass.ts(bc, 512)])
            if ib < IB - 1:
                nc.gpsimd.dma_start(out=out[:, ib], in_=o_t)
            # release the fp16 x loads for later k-pairs only after the
            # matching eviction round, so they can't starve early DMAs.
            # (trigger rides the ACT instruction stream: it executes only
            # once the preceding activations have issued)
            if ib in (1, 3, 5):
                tnext = ib // 2 + 1
                for bc in range(NBC):
                    nc.scalar.dma_start(out=xf_t[tnext][:, bc],
                                        in_=xf[:, tnext, bc])

    nc.compile()
    return nc


def _get_nc():
    if "nc" not in _CACHE:
        _CACHE["nc"] = _build_nc()
    return _CACHE["nc"]


def _prep_inputs(x, lin_W, lin_b, W1, b1, W2):
    """Host-side prep: fold the basis chain, split W into off-diag + diag,
    quantize to fp8/fp16, and lay out transposed per core."""
    x = np.asarray(x, dtype=np.float32)
    lin_W = np.asarray(lin_W, dtype=np.float32)
    a = np.sum(np.asarray(W1, np.float32) * np.asarray(W2, np.float32),
               axis=1)
    c = np.sum(np.asarray(b1, np.float32) * np.asarray(W2, np.float32),
               axis=1)
    W_eff = np.ascontiguousarray(lin_W.T)
    idx = np.arange(D)
    W_eff[idx, idx] += a
    d = W_eff[idx, idx].copy()
    W_off = W_eff
    W_off[idx, idx] = 0.0
    bias = (np.asarray(lin_b, np.float32) + c).astype(np.float32)

    x8 = x.astype(FP8_NP)
    xf = x.astype(np.float16)
    w8 = W_off.astype(FP8_NP)

    # w8 dram layout [p, ib, t, 2, i128]: j = (2*t+sub)*128 + p,
    # i = ib*128 + i128
    w8_dev = np.ascontiguousarray(
        w8.reshape(TP, 2, P, IB, P).transpose(2, 3, 0, 1, 4))
    dv_dev = np.ascontiguousarray(d.reshape(IB, P).T)
    bv_dev = np.ascontiguousarray(bias.reshape(IB, P).T)

    def xpose(arr):  # [NCORES*BS, D] -> per-core [p, t, bc, 2, b512]
        t = arr.reshape(NCORES, NBC, 512, TP, 2, P)
        return np.ascontiguousarray(t.transpose(0, 5, 3, 1, 4, 2))

    x8_dev = xpose(x8)
    xf_dev = xpose(xf)

    return [
        {"x8": x8_dev[i], "xf": xf_dev[i], "w8": w8_dev,
         "dv": dv_dev, "bv": bv_dev}
        for i in range(NCORES)
    ]


def kernel(x, lin_W, lin_b, W1, b1, W2):
    in_maps = _prep_inputs(x, lin_W, lin_b, W1, b1, W2)
    nc = _get_nc()
    res = run_bass_kernel_spmd(nc, in_maps, core_ids=list(range(NCORES)))
    # out^T [p, ib, b] per core -> [b_global, ib*128+p]
    o = np.stack([r["out"] for r in res.results])  # [cores, P, IB, BS] bf16
    o = o.astype(np.float32).transpose(0, 3, 2, 1).reshape(B, D)
    return np.ascontiguousarray(o)


# revision 14
# speedup vs baseline: 1.0533x; 1.0533x over previous
"""Trainium2 Bass kernel for nn_KANLayer (B=16384, D=1024, K=8).

Math: the per-feature basis chain collapses algebraically:
    nl[b,i] = sum_k (x[b,i]*W1[i,k] + b1[i,k]) * W2[i,k]
            = x[b,i] * a[i] + c[i],   a = sum_k W1*W2, c = sum_k b1*W2
so the whole layer is ONE dense matmul with a fused diagonal + bias:
    out = x @ (lin_W.T + diag(a)) + (lin_b + c)

Precision strategy (validated numerically on the fixed seed-0 inputs,
rel err ~1e-2 vs the 2e-2 gate): split W_eff = W_off + diag(d).
  - x @ W_off runs on the PE in fp8 e4m3 with perf_mode=DoubleRow
    (2 fp8/cell, K=256 per matmul) — ~216ns per [256x128x512] matmul.
  - the diagonal term d[i]*x[b,i] is large (|d|~1 vs |W_off|~1/32), so
    it is applied at eviction from an fp16 copy of x with one DVE
    scalar_tensor_tensor op per tile.
  - output stored fp16 (|out|<=21, rel rounding err ~2^-11).

Layout: everything transposed — W is the PE-stationary operand, x^T the
moving one, so psum holds out^T[i, b] and the diagonal/bias become
per-partition scalars. Host transposes the output back.

DMA scheduling: each dma_start lands on one HW queue and all enqueued
transfers fair-share HBM bandwidth, so loads are split into pieces and
issued in priority order: tiny 64KB partition-split pieces for the
very first matmul's operands, then the rest of x8/w8, then fp16 x for
k-pair 0; fp16 x for k-pairs 1-3 is released from the ACT instruction
stream after earlier evictions. Stores ride the gpsimd ring.

Sharding: data-parallel over batch across 8 NeuronCores (2048 rows
each); weights replicated. No collectives.
"""

from contextlib import ExitStack

import numpy as np
import ml_dtypes

import concourse.bass as bass
import concourse.tile as tile
from concourse import bacc, mybir
from concourse.bass_utils import run_bass_kernel_spmd

B, D = 16384, 1024
NCORES = 8
BS = B // NCORES   # 2048 batch rows per core
P = 128
TP = 4             # contraction k-pairs (each pair = 256 rows via DoubleRow)
IB = D // P        # 8 output-feature blocks of 128
NBC = BS // 512    # 4 batch chunks of 512

FP8 = mybir.dt.float8e4
FP8_NP = ml_dtypes.float8_e4m3fn  # matches TRN fp8e4 within +-240
F16 = mybir.dt.float16

_CACHE = {}


def _build_nc():
    nc = bacc.Bacc("TRN2", target_bir_lowering=False, debug=False,
                   num_devices=NCORES)
    # x8: x^T per core in fp8 (PE moving operand), layout [p, t, bc, 2, b512]
    # with contraction index j = (2*t + sub)*128 + p; each (t, bc) piece is
    # contiguous per partition.
    x8 = nc.dram_tensor("x8", [P, TP, NBC, 2, 512], FP8,
                        kind="ExternalInput").ap()
    # xf: x^T in fp16 for the diagonal correction, same layout.
    xf = nc.dram_tensor("xf", [P, TP, NBC, 2, 512], F16,
                        kind="ExternalInput").ap()
    # W_off (diag zeroed), layout [p, ib, t, 2, i128]: one piece per ib
    # column block holding all k-pairs.
    w8 = nc.dram_tensor("w8", [P, IB, TP, 2, P], FP8,
                        kind="ExternalInput").ap()
    # diag + bias as per-partition columns: dv[p, ib] = d[ib*128+p]
    dv = nc.dram_tensor("dv", [P, IB], mybir.dt.float32,
                        kind="ExternalInput").ap()
    bv = nc.dram_tensor("bv", [P, IB], mybir.dt.float32,
                        kind="ExternalInput").ap()
    # out^T fp16: out[p, ib, b] = result[b, ib*128+p]
    out = nc.dram_tensor("out", [P, IB, BS], F16, kind="ExternalOutput").ap()

    Act = mybir.ActivationFunctionType
    Alu = mybir.AluOpType
    DR = mybir.MatmulPerfMode.DoubleRow

    with tile.TileContext(nc) as tc, ExitStack() as ctx:
        cpool = ctx.enter_context(tc.tile_pool(name="cpool", bufs=1))
        opool = ctx.enter_context(tc.tile_pool(name="opool", bufs=3))
        tpool = ctx.enter_context(tc.tile_pool(name="tpool", bufs=8))
        ppool = ctx.enter_context(tc.tile_pool(name="ppool", bufs=8,
                                               space="PSUM"))

        x_t = [cpool.tile([P, NBC, 2, 512], FP8, tag=f"x{t}", name=f"x_t{t}")
               for t in range(TP)]
        w_t = cpool.tile([P, IB, TP, 2, P], FP8, tag="w", name="w_t")

        # --- priority wave 0: first matmul's operands as 64KB
        # partition-split pieces ---
        for h in range(2):
            hp = bass.ts(h, 64)
            nc.sync.dma_start(out=x_t[0][hp, 0], in_=x8[hp, 0, 0])
            nc.scalar.dma_start(out=w_t[hp, 0], in_=w8[hp, 0])
        # --- wave 1: rest of x8 k-pair 0 + rest of w8 ---
        for bc in range(1, NBC):
            nc.sync.dma_start(out=x_t[0][:, bc], in_=x8[:, 0, bc])
        for ib in range(1, IB):
            nc.scalar.dma_start(out=w_t[:, ib], in_=w8[:, ib])

        dv_t = cpool.tile([P, IB], mybir.dt.float32, tag="dv", name="dv_t")
        bv_t = cpool.tile([P, IB], mybir.dt.float32, tag="bv", name="bv_t")
        nc.sync.dma_start(out=dv_t, in_=dv)
        nc.sync.dma_start(out=bv_t, in_=bv)

        # --- wave 2: rest of x8 (bc-pair pieces), then fp16 x k-pair 0 ---
        xf_t = [cpool.tile([P, NBC, 2, 512], F16, tag=f"xf{t}",
                           name=f"xf_t{t}") for t in range(TP)]
        for t in range(1, TP):
            for b2 in range(NBC // 2):
                nc.sync.dma_start(out=x_t[t][:, bass.ts(b2, 2)],
                                  in_=x8[:, t, bass.ts(b2, 2)])
        for b2 in range(NBC // 2):
            nc.scalar.dma_start(out=xf_t[0][:, bass.ts(b2, 2)],
                                in_=xf[:, 0, bass.ts(b2, 2)])

        # PE pre-warm with fp8 DoubleRow matmuls (tile zeroed on the idle
        # gpsimd engine so warmup starts as soon as the PE boots).
        warm = cpool.tile([P, 2, 512], FP8, tag="warm", name="warm")
        nc.gpsimd.memset(warm, 0.0)
        warm_ps = ppool.tile([P, 512], mybir.dt.float32, tag="ps",
                             name="warm_ps")
        NWARM = 8
        for i in range(NWARM):
            nc.tensor.matmul(warm_ps, lhsT=warm[:, :, :P], rhs=warm,
                             start=(i == 0), stop=(i == NWARM - 1),
                             perf_mode=DR)

        for ib in range(IB):
            psums = [ppool.tile([P, 512], mybir.dt.float32, tag="ps",
                                name=f"ps{ib}_{bc}") for bc in range(NBC)]
            for t in range(TP):
                for bc in range(NBC):
                    nc.tensor.matmul(
                        psums[bc],
                        lhsT=w_t[:, ib, t],
                        rhs=x_t[t][:, bc],
                        start=(t == 0),
                        stop=(t == TP - 1),
                        perf_mode=DR,
                    )
            # eviction: out^T[i,b] = psum + d[i]*x_f16[i,b] + bias[i]
            t8, sub = divmod(ib, 2)
            o_t = opool.tile([P, BS], F16, tag="o", name=f"o_t{ib}")
            for bc in range(NBC):
                tb = tpool.tile([P, 512], F16, tag="tb",
                                name=f"tb{ib}_{bc}")
                nc.scalar.activation(tb, psums[bc], Act.Identity,
                                     bias=bv_t[:, ib:ib + 1], scale=1.0)
                nc.vector.scalar_tensor_tensor(
                    o_t[:, bass.ts(bc, 512)], in0=xf_t[t8][:, bc, sub],
                    scalar=dv_t[:, ib:ib + 1],
                    in1=tb, op0=Alu.mult, op1=Alu.add)
                if ib == IB - 1:
                    # pipeline the kernel tail: store per 512-chunk
                    nc.gpsimd.dma_start(out=out[:, ib, bass.ts(bc, 512)],
                                        in_=o_t[:, bass.ts(bc, 512)])
            if ib < IB - 1:
                nc.gpsimd.dma_start(out=out[:, ib], in_=o_t)
            # release the fp16 x loads for later k-pairs only after the
            # matching eviction round, so they can't starve early DMAs.
            # (trigger rides the ACT instruction stream: it executes only
            # once the preceding activations have issued)
            if ib in (1, 3, 5):
                tnext = ib // 2 + 1
                for b2 in range(NBC // 2):
                    nc.scalar.dma_start(out=xf_t[tnext][:, bass.ts(b2, 2)],
                                        in_=xf[:, tnext, bass.ts(b2, 2)])

    nc.compile()
    return nc


def _get_nc():
    if "nc" not in _CACHE:
        _CACHE["nc"] = _build_nc()
    return _CACHE["nc"]


def _prep_inputs(x, lin_W, lin_b, W1, b1, W2):
    """Host-side prep: fold the basis chain, split W into off-diag + diag,
    quantize to fp8/fp16, and lay out transposed per core."""
    x = np.asarray(x, dtype=np.float32)
    lin_W = np.asarray(lin_W, dtype=np.float32)
    a = np.sum(np.asarray(W1, np.float32) * np.asarray(W2, np.float32),
               axis=1)
    c = np.sum(np.asarray(b1, np.float32) * np.asarray(W2, np.float32),
               axis=1)
    W_eff = np.ascontiguousarray(lin_W.T)
    idx = np.arange(D)
    W_eff[idx, idx] += a
    d = W_eff[idx, idx].copy()
    W_off = W_eff
    W_off[idx, idx] = 0.0
    bias = (np.asarray(lin_b, np.float32) + c).astype(np.float32)

    x8 = x.astype(FP8_NP)
    xf = x.astype(np.float16)
    w8 = W_off.astype(FP8_NP)

    # w8 dram layout [p, ib, t, 2, i128]: j = (2*t+sub)*128 + p,
    # i = ib*128 + i128
    w8_dev = np.ascontiguousarray(
        w8.reshape(TP, 2, P, IB, P).transpose(2, 3, 0, 1, 4))
    dv_dev = np.ascontiguousarray(d.reshape(IB, P).T)
    bv_dev = np.ascontiguousarray(bias.reshape(IB, P).T)

    def xpose(arr):  # [NCORES*BS, D] -> per-core [p, t, bc, 2, b512]
        t = arr.reshape(NCORES, NBC, 512, TP, 2, P)
        return np.ascontiguousarray(t.transpose(0, 5, 3, 1, 4, 2))

    x8_dev = xpose(x8)
    xf_dev = xpose(xf)

    return [
        {"x8": x8_dev[i], "xf": xf_dev[i], "w8": w8_dev,
         "dv": dv_dev, "bv": bv_dev}
        for i in range(NCORES)
    ]


def kernel(x, lin_W, lin_b, W1, b1, W2):
    in_maps = _prep_inputs(x, lin_W, lin_b, W1, b1, W2)
    nc = _get_nc()
    res = run_bass_kernel_spmd(nc, in_maps, core_ids=list(range(NCORES)))
    # out^T [p, ib, b] per core -> [b_global, ib*128+p]
    o = np.stack([r["out"] for r in res.results])  # [cores, P, IB, BS] fp16
    o = o.astype(np.float32).transpose(0, 3, 2, 1).reshape(B, D)
    return np.ascontiguousarray(o)
